# revision 12
# baseline (speedup 1.0000x reference)
"""Hawk (RG-LRU recurrent block) Trainium2 kernel, 8-core SPMD — v2.

Sharding: data-parallel over B (2 groups of 4 cores) x sequence-parallel over T
(4 chunks of 1024 tokens per batch element). The diagonal linear recurrence is
computed chunk-locally with tensor_tensor_scan, stitched across cores with one
small AllGather of per-chunk scan summaries, and corrected locally.

v2 changes vs v1 (TimelineSim 412us -> 263us, PE occupancy 71% -> 94%):
  - all matmul streams in bf16 with host-side contiguous packed weight
    layouts (2-6KB DMA descriptor rows instead of 512B transposing gathers;
    28 weight DMAs instead of 56, x loaded in 3 DMAs, consts in 3)
  - ACT-table-coherent op batching via scheduler ordering edges:
    sigmoid / exp / sqrt grouped per 3-chunk window, conv tap0+bias on the
    ACT engine so DVE stops pacing phase 1
  - no P cumprod round trip through DRAM: everything stays in SBUF
  - collective issued right after the last scan summary, fully hidden under
    the gate-half projection; pre-collective q = gelu*P, r = gelu*h_loc so
    the post-collective tail is one fused scalar_tensor_tensor per chunk
  - out-projection accumulates gc-outer into 3/3/2 psum groups (drains
    overlap the next group), persistent prefix psum tile gives phase 1
    three rotating psum slots.

fp32 is kept where precision requires: alpha, alpha^2 (1-a^2 cancellation),
scan internal state (hardware keeps fp32 state regardless of output dtype),
PSUM accumulation.
"""
import os

os.environ.setdefault("JAX_COMPILATION_CACHE_DIR", "/tmp/jax_cache_hawk")

import numpy as np

import concourse.bacc as bacc
import concourse.mybir as mybir
import concourse.tile as tile
from concourse.tile_rust import add_dep_helper
from concourse.bass_utils import run_bass_kernel_spmd

F32 = mybir.dt.float32
BF16 = mybir.dt.bfloat16
AF = mybir.ActivationFunctionType
OP = mybir.AluOpType

DIM = 1024
HID = 1536
KCONV = 4
B = 2
T = 4096
C_CONST = 8.0

NCORE = 8
TC = T // 4          # tokens per core
NH = HID // 128      # 12 hidden chunks
NDC = DIM // 128     # 8 dim chunks
PAD = 4              # left pad (>= KCONV-1, kept even for bf16 alignment)
TPAD = TC + PAD      # 1028

_CACHE: dict = {}


def _build(dbg=False, n_body=1):
    nc = bacc.Bacc("TRN2", target_bir_lowering=False, debug=False,
                   num_devices=NCORE, dynamic_dma_scratch_size=8192)

    xt = nc.dram_tensor("xt", [128, NDC * TPAD], BF16, kind="ExternalInput").ap()
    wproj = nc.dram_tensor("wproj", [12, 128, 2 * NDC * 128], BF16,
                           kind="ExternalInput").ap()
    wgates = nc.dram_tensor("wgates", [NH, 128, 2 * NH * 128], BF16,
                            kind="ExternalInput").ap()
    wout = nc.dram_tensor("wout", [4, 128, 2 * NH * 128], BF16,
                          kind="ExternalInput").ap()
    convw = nc.dram_tensor("convw", [128, NH * 5], F32, kind="ExternalInput").ap()
    gvecs = nc.dram_tensor("gvecs", [128, NH * 3], F32, kind="ExternalInput").ap()
    sel = nc.dram_tensor("sel", [128, 3], F32, kind="ExternalInput").ap()
    out = nc.dram_tensor("out", [NDC, 128, TC], F32, kind="ExternalOutput").ap()
    if dbg:
        d_uc = nc.dram_tensor("d_uc", [NH, 128, TC], F32, kind="ExternalOutput").ap()
        d_sg = nc.dram_tensor("d_sg", [NH, 128, TC], F32, kind="ExternalOutput").ap()
        d_al = nc.dram_tensor("d_al", [NH, 128, TC], F32, kind="ExternalOutput").ap()
        d_h = nc.dram_tensor("d_h", [NH, 128, TC], F32, kind="ExternalOutput").ap()
        d_p = nc.dram_tensor("d_p", [NH, 128, TC], F32, kind="ExternalOutput").ap()
        d_G = nc.dram_tensor("d_G", [128, 96], F32, kind="ExternalOutput").ap()
        d_c = nc.dram_tensor("d_c", [128, NH], F32, kind="ExternalOutput").ap()

    with tile.TileContext(nc) as tc:
      for _body_i in range(n_body):
        with (
            tc.tile_pool(name="wt", bufs=4) as wtp,
            tc.tile_pool(name="ucp", bufs=NH) as ucp,
            tc.tile_pool(name="upre", bufs=2) as uprep,
            tc.tile_pool(name="uacc", bufs=2) as uaccp,
            tc.tile_pool(name="sg", bufs=9) as sgp,
            tc.tile_pool(name="xb", bufs=9) as xbp,
            tc.tile_pool(name="al", bufs=5) as alp,
            tc.tile_pool(name="b2", bufs=3) as b2p,
            tc.tile_pool(name="hp", bufs=NH) as hp,
            tc.tile_pool(name="pp", bufs=NH) as ppp,
            tc.tile_pool(name="gg", bufs=4) as ggp,
            tc.tile_pool(name="ot", bufs=2) as otp,
            tc.tile_pool(name="cst", bufs=1) as cst,
            tc.tile_pool(name="ps", bufs=4, space="PSUM") as ps,
            tc.tile_pool(name="dram", bufs=1, space="DRAM") as dram,
        ):
            def load_w(src, ncols):
                wt = wtp.tile([128, 2 * NH * 128], BF16, tag="w", name="wt")
                nc.sync.dma_start(wt[:, 0:ncols], src)
                return wt

            # first weight half-pair + first x^T chunk start the PE asap
            w_first = wtp.tile([128, 2 * NH * 128], BF16, tag="w", name="wt")
            nc.sync.dma_start(w_first[:, 0:NDC * 128], wproj[0][:, 0:NDC * 128])
            xt_all = cst.tile([128, NDC * TPAD], BF16, tag="xt", name="xt_all")
            nc.sync.dma_start(xt_all[:, 0:TPAD], xt[:, 0:TPAD])
            nc.sync.dma_start(w_first[:, NDC * 128:2 * NDC * 128],
                              wproj[0][:, NDC * 128:2 * NDC * 128])
            nc.sync.dma_start(xt_all[:, TPAD:4 * TPAD], xt[:, TPAD:4 * TPAD])
            nc.sync.dma_start(xt_all[:, 4 * TPAD:], xt[:, 4 * TPAD:])
            xt_t = [xt_all[:, cc * TPAD:(cc + 1) * TPAD] for cc in range(NDC)]

            # ---- constants (3 merged DMAs; only needed once conv starts) ----
            convw_t = cst.tile([128, NH * 5], F32, tag="cw", name="convw_t")
            nc.sync.dma_start(convw_t[:], convw[:])
            gvecs_t = cst.tile([128, NH * 3], F32, tag="gv", name="gvecs_t")
            nc.sync.dma_start(gvecs_t[:], gvecs[:])
            sel_t = cst.tile([128, 3], F32, tag="sel", name="sel_t")
            nc.sync.dma_start(sel_t[:], sel[:])
            S_loc = cst.tile([128, 24], F32, tag="sloc", name="S_loc")
            G_t = cst.tile([128, 96], F32, tag="gt", name="G_t")
            p2_t = cst.tile([128, NH], F32, tag="p2", name="p2_t")
            p3_t = cst.tile([128, NH], F32, tag="p3", name="p3_t")
            c_t = cst.tile([128, NH], F32, tag="ct", name="c_t")

            def proj_matmuls(pt, p3t, wt):
                # main: tokens 0..1023 live at columns PAD..TPAD
                for th in range(2):
                    for cc in range(NDC):
                        nc.tensor.matmul(
                            pt[:, th * 512:(th + 1) * 512],
                            wt[:, cc * 128:(cc + 1) * 128],
                            xt_t[cc][:, PAD + th * 512: PAD + (th + 1) * 512],
                            start=(cc == 0), stop=(cc == NDC - 1))
                if p3t is not None:
                    # prefix tokens -4..-1 (columns 0..3); only -3..-1 used
                    for cc in range(NDC):
                        nc.tensor.matmul(
                            p3t[:, 0:4],
                            wt[:, cc * 128:(cc + 1) * 128],
                            xt_t[cc][:, 0:4],
                            start=(cc == 0), stop=(cc == NDC - 1))

            # ---- phase 1: u half of proj + causal conv ----
            # one persistent psum tile holds all 12 prefix regions, so the
            # main pt tiles get 3 rotating slots (deeper PE pipelining)
            p3all = ps.tile([128, 4 * NH], F32, tag="ps", name="p3all")
            u_c = []
            for mc in range(NH):
                if mc % 2 == 0:
                    wt_pair = (w_first if mc == 0
                               else load_w(wproj[mc // 2], 2 * NDC * 128))
                wt = wt_pair[:, (mc % 2) * NDC * 128:(mc % 2 + 1) * NDC * 128]
                pt = ps.tile([128, TC], F32, tag="ps", name="pt")
                p3t = p3all[:, mc * 4:(mc + 1) * 4]
                proj_matmuls(pt, p3t, wt)
                upre = uprep.tile([128, TPAD], F32, tag="upre", name="upre")
                nc.scalar.copy(upre[:, PAD:TPAD], pt[:])
                nc.vector.tensor_copy(upre[:, 0:4], p3t)
                w5 = convw_t[:, mc * 5:(mc + 1) * 5]
                # u_c[t] = sum_k w_k * u_pre[t-3+k] + conv_b
                # upre col c = token c - PAD, so tap k reads cols k+1..k+1+TC
                # tap 0 + bias ride the (otherwise idle) ACT engine
                uacc = uaccp.tile([128, TC], F32, tag="uacc", name="uacc")
                nc.scalar.activation(uacc[:], upre[:, 1:1 + TC], AF.Identity,
                                     bias=w5[:, 4:5], scale=w5[:, 0:1])
                for k in range(1, KCONV - 1):
                    nc.vector.scalar_tensor_tensor(
                        uacc[:], upre[:, k + 1:k + 1 + TC], w5[:, k:k + 1],
                        uacc[:], OP.mult, OP.add)
                uc = ucp.tile([128, TC], BF16, tag="uc", name="uc")
                nc.vector.scalar_tensor_tensor(
                    uc[:], upre[:, KCONV:KCONV + TC],
                    w5[:, KCONV - 1:KCONV], uacc[:], OP.mult, OP.add)
                if dbg:
                    ducf = otp.tile([128, TC], F32, tag="ot", name="ducf")
                    nc.vector.tensor_copy(ducf[:], uc[:])
                    nc.sync.dma_start(d_uc[mc], ducf[:])
                u_c.append(uc)

            # ---- phase 2: gates + scans, ACT-table-coherent windows ----
            h_t: list = [None] * NH
            P_t: list = [None] * NH

            def emit_sig(gc, SG, XI):
                gv = gvecs_t[:, gc * 3:(gc + 1) * 3]
                pig = ps.tile([128, TC], F32, tag="ps", name="pig")
                prg = ps.tile([128, TC], F32, tag="ps", name="prg")
                wt = load_w(wgates[gc], 2 * NH * 128)
                for dst, wi in ((pig, 0), (prg, 1)):
                    base = wi * NH * 128
                    for th in range(2):
                        for hc in range(NH):
                            nc.tensor.matmul(
                                dst[:, th * 512:(th + 1) * 512],
                                wt[:, base + hc * 128:base + (hc + 1) * 128],
                                u_c[hc][:, th * 512:(th + 1) * 512],
                                start=(hc == 0), stop=(hc == NH - 1))
                sg = sgp.tile([128, TC], BF16, tag="sg", name="sg")
                i1 = nc.scalar.activation(sg[:], prg[:], AF.Sigmoid,
                                          bias=gv[:, 2:3])
                xi = xbp.tile([128, TC], BF16, tag="xb", name="xi")
                i2 = nc.scalar.activation(xi[:], pig[:], AF.Sigmoid,
                                          bias=gv[:, 1:2])
                nc.vector.tensor_tensor(xi[:], xi[:], u_c[gc][:], OP.mult)
                if dbg:
                    dsgf = otp.tile([128, TC], F32, tag="ot", name="dsgf")
                    nc.vector.tensor_copy(dsgf[:], sg[:])
                    nc.sync.dma_start(d_sg[gc], dsgf[:])
                SG[gc] = sg
                XI[gc] = xi
                return i1, i2

            def emit_exp(gc, SG, AL):
                gv = gvecs_t[:, gc * 3:(gc + 1) * 3]
                al = alp.tile([128, TC], F32, tag="al", name="al")
                i1 = nc.scalar.activation(al[:], SG[gc][:], AF.Exp,
                                          scale=gv[:, 0:1])
                if dbg:
                    nc.sync.dma_start(d_al[gc], al[:])
                AL[gc] = al
                return i1

            def emit_scan(gc, XI, AL):
                xi, al = XI[gc], AL[gc]
                b2 = b2p.tile([128, TC], F32, tag="b2", name="b2")
                # Square lives in every ACT table (incl. sqrt's): no reload
                nc.scalar.activation(b2[:], al[:], AF.Square)
                nc.vector.tensor_scalar(b2[:], b2[:], -1.0, 1.000001,
                                        OP.mult, OP.add)
                isq = nc.scalar.activation(b2[:], b2[:], AF.Sqrt)
                nc.vector.tensor_tensor(xi[:], xi[:], b2[:], OP.mult)
                hl = hp.tile([128, TC], BF16, tag="h", name="hl")
                nc.vector.tensor_tensor_scan(
                    hl[:], al[:], xi[:], 0.0, OP.mult, OP.add)
                pp = ppp.tile([128, TC], BF16, tag="p", name="pp")
                nc.vector.tensor_tensor_scan(
                    pp[:], al[:], al[:], 1.0, OP.mult, OP.bypass)
                nc.vector.tensor_copy(S_loc[:, gc:gc + 1], pp[:, TC - 1:TC])
                nc.vector.tensor_copy(S_loc[:, 12 + gc:13 + gc],
                                      hl[:, TC - 1:TC])
                if dbg:
                    dhf = otp.tile([128, TC], F32, tag="ot", name="dhf")
                    nc.vector.tensor_copy(dhf[:], hl[:])
                    nc.sync.dma_start(d_h[gc], dhf[:])
                    dpf = otp.tile([128, TC], F32, tag="ot", name="dpf")
                    nc.vector.tensor_copy(dpf[:], pp[:])
                    nc.sync.dma_start(d_p[gc], dpf[:])
                h_t[gc] = hl
                P_t[gc] = pp
                return isq

            # windows of 4 chunks: sigmoids stream with the matmuls; exp /
            # square / sqrt / scans of the window run while the next window's
            # matmuls occupy the PE. 3 table loads per window.
            SG: dict = {}
            XI: dict = {}
            AL: dict = {}
            windows = [range(0, 5), range(5, 9), range(9, 12)]

            def emit_group(gcs, SG, XI, AL, last_sig):
                """exp group, then sqrt group, with ordering edges."""
                exps = [emit_exp(gc, SG, AL) for gc in gcs]
                add_dep_helper(exps[0].ins, last_sig.ins,
                               reason="ACT table batching: exp after sigmoids")
                sqrts = [emit_scan(gc, XI, AL) for gc in gcs]
                add_dep_helper(sqrts[0].ins, exps[-1].ins,
                               reason="ACT table batching: sqrt after exps")
                return sqrts[-1]

            pend = None
            last_sig = None
            last_sqrt = None
            for wi, w in enumerate(windows):
                last_window = wi == len(windows) - 1
                if pend is not None:
                    last_sqrt = emit_group(pend, SG, XI, AL, last_sig)
                sigs = []
                for gc in w:
                    sigs.extend(emit_sig(gc, SG, XI))
                    if last_window:
                        # last window: drain per-chunk so the collective's
                        # inputs finish while the gate matmuls still run;
                        # the table thrash hides under the PE window.
                        emit_exp(gc, SG, AL)
                        last_sqrt = emit_scan(gc, XI, AL)
                if not last_window and last_sqrt is not None:
                    add_dep_helper(sigs[0].ins, last_sqrt.ins,
                                   reason="ACT table batching: sig after sqrt")
                last_sig = sigs[-1]
                pend = w

            # ---- collective: gather (A, b) summaries within batch group ----
            cin = dram.tile([128, 24], F32, tag="cin", name="cin")
            cout = dram.tile([4, 128, 24], F32, tag="cout", name="cout")
            nc.sync.dma_start(cin[:], S_loc[:])
            nc.gpsimd.collective_compute(
                "AllGather", OP.bypass,
                replica_groups=[[0, 1, 2, 3], [4, 5, 6, 7]],
                ins=[cin.opt()], outs=[cout.opt()])

            # ---- phase 2.5 (fills the gather gap): gate proj + gelu + q,r --
            for i in range(NH):
                if i % 2 == 0:
                    wt_pair = load_w(wproj[(NH + i) // 2], 2 * NDC * 128)
                wt = wt_pair[:, (i % 2) * NDC * 128:(i % 2 + 1) * NDC * 128]
                pt = ps.tile([128, TC], F32, tag="ps", name="pt")
                proj_matmuls(pt, None, wt)
                gg = ggp.tile([128, TC], BF16, tag="gg", name="gg")
                ig = nc.scalar.activation(gg[:], pt[:], AF.Gelu)
                if i == 0:
                    add_dep_helper(ig.ins, last_sqrt.ins,
                                   reason="ACT table batching: gelu after sqrt")
                # q = gelu*P (into P), r = gelu*h_loc (into h)
                nc.vector.tensor_tensor(P_t[i][:], gg[:], P_t[i][:], OP.mult)
                nc.vector.tensor_tensor(h_t[i][:], gg[:], h_t[i][:], OP.mult)

            # ---- carry composition ----
            for r in range(4):
                nc.sync.dma_start(G_t[:, r * 24:(r + 1) * 24], cout[r])
            p1 = G_t[:, 12:24]
            nc.vector.tensor_tensor(p2_t[:], G_t[:, 24:36], p1, OP.mult)
            nc.vector.tensor_tensor(p2_t[:], p2_t[:], G_t[:, 36:48], OP.add)
            nc.vector.tensor_tensor(p3_t[:], G_t[:, 48:60], p2_t[:], OP.mult)
            nc.vector.tensor_tensor(p3_t[:], p3_t[:], G_t[:, 60:72], OP.add)
            nc.vector.tensor_scalar(c_t[:], p1, sel_t[:, 0:1], None, OP.mult)
            nc.vector.scalar_tensor_tensor(c_t[:], p2_t[:], sel_t[:, 1:2],
                                           c_t[:], OP.mult, OP.add)
            nc.vector.scalar_tensor_tensor(c_t[:], p3_t[:], sel_t[:, 2:3],
                                           c_t[:], OP.mult, OP.add)

            if dbg:
                nc.sync.dma_start(d_G[:], G_t[:])
                nc.sync.dma_start(d_c[:], c_t[:])

            # ---- phase 3: gh = q*c + r, out projection (gc-outer) ----
            for gc in range(NH):
                nc.vector.scalar_tensor_tensor(
                    P_t[gc][:], P_t[gc][:], c_t[:, gc:gc + 1],
                    h_t[gc][:], OP.mult, OP.add)

            # 3/3/2 groups: one psum slot stays free, so the next group's
            # first accumulation starts while this group's psums drain
            wpairs = [load_w(wout[j], 2 * NH * 128) for j in range(4)]
            wts_all = [wpairs[dc // 2][:, (dc % 2) * NH * 128:
                                      (dc % 2 + 1) * NH * 128]
                       for dc in range(NDC)]
            for group in ([0, 1, 2], [3, 4, 5], [6, 7]):
                pos = [ps.tile([128, TC], F32, tag="ps", name=f"po{dc}")
                       for dc in group]
                for gc in range(NH):
                    for i, dc in enumerate(group):
                        for th in range(2):
                            nc.tensor.matmul(
                                pos[i][:, th * 512:(th + 1) * 512],
                                wts_all[dc][:, gc * 128:(gc + 1) * 128],
                                P_t[gc][:, th * 512:(th + 1) * 512],
                                start=(gc == 0), stop=(gc == NH - 1))
                for i, dc in enumerate(group):
                    ot = otp.tile([128, TC], F32, tag="ot", name="ot")
                    nc.scalar.copy(ot[:], pos[i][:])
                    nc.sync.dma_start(out[dc], ot[:])

    nc.compile()
    return nc


def _softplus64(x):
    x = np.asarray(x, np.float64)
    return np.log1p(np.exp(-np.abs(x))) + np.maximum(x, 0.0)


def _prepare(x, W_proj, conv_w, conv_b, W_in, b_in, W_gate, b_gate,
             forget_lambda, W_out):
    bf16 = mybir.dt.np(BF16)
    x = np.asarray(x, np.float32)
    W_proj = np.asarray(W_proj, np.float32)
    conv_w = np.asarray(conv_w, np.float32)
    conv_b = np.asarray(conv_b, np.float32)
    b_in = np.asarray(b_in, np.float32)
    b_gate = np.asarray(b_gate, np.float32)
    forget_lambda = np.asarray(forget_lambda, np.float32)

    # wproj rows: mc 0..11 = u rows (1536:3072), mc 12..23 = gate rows (0:1536)
    # layout [mc, k, c*128+m] = W_proj[row0+m, c*128+k]
    order = list(range(12, 24)) + list(range(0, 12))
    wp_ = W_proj.reshape(24, 128, NDC, 128)[order]
    wproj_flat = wp_.transpose(0, 3, 2, 1).reshape(24, 128, NDC * 128)
    # pack consecutive mc pairs side by side: [12, 128, 2048]
    wproj = np.ascontiguousarray(
        wproj_flat.reshape(12, 2, 128, NDC * 128).transpose(0, 2, 1, 3)
        .reshape(12, 128, 2 * NDC * 128)).astype(bf16)

    def gate_layout(W):
        w_ = np.asarray(W, np.float32).reshape(NH, 128, NH, 128)
        return w_.transpose(0, 3, 2, 1).reshape(NH, 128, NH * 128)

    # both gate matrices of one chunk side by side: [12, 128, 3072]
    wgates = np.ascontiguousarray(np.concatenate(
        [gate_layout(W_in), gate_layout(W_gate)], axis=2)).astype(bf16)

    wo_ = np.asarray(W_out, np.float32).reshape(NDC, 128, NH, 128)
    wout_flat = wo_.transpose(0, 3, 2, 1).reshape(NDC, 128, NH * 128)
    wout = np.ascontiguousarray(
        wout_flat.reshape(4, 2, 128, NH * 128).transpose(0, 2, 1, 3)
        .reshape(4, 128, 2 * NH * 128)).astype(bf16)

    convw_ = np.concatenate(
        [conv_w[:, 0, :].reshape(NH, 128, KCONV),
         conv_b.reshape(NH, 128, 1)], axis=2)          # [hc, k, 5]
    convw = np.ascontiguousarray(
        convw_.transpose(1, 0, 2).reshape(128, NH * 5)).astype(np.float32)

    negrate = (-C_CONST * _softplus64(forget_lambda)).astype(np.float32)
    gvecs_ = np.stack(
        [negrate.reshape(NH, 128),
         b_in.reshape(NH, 128),
         b_gate.reshape(NH, 128)], axis=2)             # [hc, k, 3]
    gvecs = np.ascontiguousarray(
        gvecs_.transpose(1, 0, 2).reshape(128, NH * 3)).astype(np.float32)

    in_maps = []
    for c in range(NCORE):
        bb, j = divmod(c, 4)
        lo = j * TC - PAD
        if lo < 0:
            chunk = np.concatenate(
                [np.zeros((PAD, DIM), np.float32), x[bb, 0:(j + 1) * TC]])
        else:
            chunk = x[bb, lo:(j + 1) * TC]
        xtc = np.ascontiguousarray(
            chunk.T.reshape(NDC, 128, TPAD).transpose(1, 0, 2)
            .reshape(128, NDC * TPAD)).astype(bf16)
        selc = np.zeros((128, 3), np.float32)
        if j > 0:
            selc[:, j - 1] = 1.0
        in_maps.append({
            "xt": xtc, "wproj": wproj, "wgates": wgates, "wout": wout,
            "convw": convw, "gvecs": gvecs, "sel": selc,
        })
    return in_maps


def _get_nc():
    if "nc" not in _CACHE:
        _CACHE["nc"] = _build()
    return _CACHE["nc"]


def _build_debug():
    return _build(dbg=True)


def kernel(x, W_proj, conv_w, conv_b, W_in, b_in, W_gate, b_gate,
           forget_lambda, W_out):
    nc = _get_nc()
    in_maps = _prepare(x, W_proj, conv_w, conv_b, W_in, b_in, W_gate, b_gate,
                       forget_lambda, W_out)
    try:
        res = run_bass_kernel_spmd(nc, in_maps, core_ids=list(range(NCORE)))
    except Exception:
        # one retry: the axon-tunneled device occasionally reports a
        # transient unrecoverable state that clears on reconnect
        import time as _time
        _time.sleep(10)
        res = run_bass_kernel_spmd(nc, in_maps, core_ids=list(range(NCORE)))
    out = np.empty((B, T, DIM), np.float32)
    for c in range(NCORE):
        bb, j = divmod(c, 4)
        o = res.results[c]["out"].reshape(DIM, TC)
        out[bb, j * TC:(j + 1) * TC, :] = o.T
    return out
